# revision 15
# baseline (speedup 1.0000x reference)
"""KWinnersCompetition forward kernel for 8 Trainium2 NeuronCores.

The reference's top-k mask only gates gradients (where(mask, x, stop_grad(x))
has forward value x), so the forward output is exactly:

    out[b, c, h, w] = relu(x[b, c, h, w] - mean_c' x[b, c', h, w])

Sharding: data-parallel over batch. 64 batches / 8 cores = 8 per core,
no communication.

The kernel is purely memory-bound (roofline = HBM traffic / ~430 GB/s
per core), and the tolerance is 2e-2, so the single biggest lever is
moving bf16 instead of f32 across HBM: the host downcasts x to bf16
before upload and upcasts y back to f32 after download, halving the
mandatory traffic (25.7 MB -> 12.85 MB per core). bf16 rounding of x
costs ~2^-9 relative error (~2e-3 of the output max) - an order of
magnitude inside tolerance. It also makes the PE mean input bf16
natively, so no cast op is needed on any engine.

Per-core layout (x shard [8, 512, 784] bf16, C-major so HW is
contiguous). Channels are interleaved onto partitions as c = 4p + j
(partition p, free-dim j in 0..3) so every partition's DMA run is
contiguous DRAM.

DMA plan: ALL transfers (16 half-batch loads, then 16 half-batch
stores) are issued from the Sync engine onto its single HWDGE ring.
Ring FIFO order = issue order, so every load descriptor drains before
any store descriptor: loads get strict priority (every load is on the
critical path of downstream compute; stores only gate the very end).
Earlier two-ring variants let the store ring steal SDMA bandwidth
mid-stream, making the last loads dribble out ~6 us late. Store
dma_starts wait on their relu sems on the otherwise-idle Sync
sequencer, so they can never block compute engines either.

Compute per batch (halves of 392 columns = one PSUM bank):
  - PE:  per half, 4 accumulating bf16 matmuls with a constant 1/512
    weight tile: m = (1/512) * sum_c x[c, :] broadcast to all 128
    partitions (f32 PSUM accumulate). The j0/j1 matmuls only need the
    first half-batch load, so PE starts ~2 us earlier than with
    full-batch loads.
  - DVE: copy the f32 PSUM mean to a bf16 SBUF tile, then one
    all-bf16 tensor_sub per half with the mean AP broadcast over j
    (all-bf16 lets DVE hit its 2x mode).
  - ACT: relu per half (bf16 -> bf16), nothing else.
"""

import sys

if "/opt/trn_rl_repo" not in sys.path:
    sys.path.insert(0, "/opt/trn_rl_repo")

import numpy as np

B, C, H, W = 64, 512, 28, 28
HW = H * W              # 784
NCORES = 8
BPC = B // NCORES       # 8 batches per core
P = 128                 # partitions
J = C // P              # 4 channels interleaved per partition
HALF = HW // 2          # 392 (matmul free dim <= 512 / one PSUM bank)

_built = None


def _build():
    import concourse.bacc as bacc
    import concourse.bass as bass
    import concourse.tile as tile
    from concourse import mybir

    nc = bacc.Bacc("TRN2", target_bir_lowering=False, debug=False)
    x = nc.dram_tensor("x", [BPC, C, HW], mybir.dt.bfloat16, kind="ExternalInput")
    y = nc.dram_tensor("y", [BPC, C, HW], mybir.dt.bfloat16, kind="ExternalOutput")

    bf16 = mybir.dt.bfloat16

    with tile.TileContext(nc) as tc:
        with (
            tc.tile_pool(name="singles", bufs=1) as singles,
            tc.tile_pool(name="xin", bufs=BPC) as xin,
            tc.tile_pool(name="diffs", bufs=6) as diffs,
            tc.tile_pool(name="outs", bufs=6) as outs,
            tc.tile_pool(name="m16s", bufs=4) as m16s,
            tc.tile_pool(name="means", bufs=4, space="PSUM") as means,
        ):
            wones = singles.tile([P, P], bf16)
            nc.vector.memset(wones, 1.0 / C)

            # 16 half-batch loads (j-pairs, contiguous per partition),
            # all on the Sync ring ahead of every store
            xts = []
            for b in range(BPC):
                xb = x[b].rearrange("(p j) w -> p j w", j=J)
                xt = xin.tile([P, J, HW], bf16)
                nc.sync.dma_start(out=xt[:, 0:2, :], in_=xb[:, 0:2, :])
                nc.sync.dma_start(out=xt[:, 2:4, :], in_=xb[:, 2:4, :])
                xts.append(xt)

            for b in range(BPC):
                yb = y[b].rearrange("(p j) w -> p j w", j=J)
                xt = xts[b]

                dt = diffs.tile([P, J, HW], bf16)
                ot = outs.tile([P, J, HW], bf16)

                for h in range(2):
                    lo = h * HALF
                    hi = lo + HALF
                    m = means.tile([P, HALF], mybir.dt.float32)
                    for j in range(J):
                        nc.tensor.matmul(
                            m,
                            wones,
                            xt[:, j, lo:hi],
                            start=(j == 0),
                            stop=(j == J - 1),
                        )
                    m16 = m16s.tile([P, HALF], bf16)
                    nc.vector.tensor_copy(out=m16, in_=m)
                    # mean AP broadcast across the j dim (step 0)
                    map_ = m16[:]
                    m_bcast = bass.AP(
                        tensor=map_.tensor,
                        offset=map_.offset,
                        ap=[map_.ap[0], [0, J], map_.ap[1]],
                    )
                    nc.vector.tensor_sub(dt[:, :, lo:hi], xt[:, :, lo:hi], m_bcast)
                    nc.scalar.activation(
                        ot[:, :, lo:hi],
                        dt[:, :, lo:hi],
                        func=mybir.ActivationFunctionType.Relu,
                    )
                    # per-half store, issued from Sync: queues on the same
                    # ring BEHIND all loads -> loads drain first
                    nc.sync.dma_start(out=yb[:, :, lo:hi], in_=ot[:, :, lo:hi])

    nc.compile()
    return nc


def _get_nc():
    global _built
    if _built is None:
        _built = _build()
    return _built


def _shard(x_full):
    import ml_dtypes

    xf = np.asarray(x_full).reshape(B, C, HW).astype(ml_dtypes.bfloat16)
    return [
        {"x": np.ascontiguousarray(xf[i * BPC : (i + 1) * BPC])}
        for i in range(NCORES)
    ]


def _run(in_maps, **kw):
    from concourse.bass_utils import run_bass_kernel_spmd

    return run_bass_kernel_spmd(_get_nc(), in_maps, list(range(NCORES)), **kw)


def kernel(x, k=None, **_unused):
    res = _run(_shard(np.asarray(x)))
    out = np.concatenate(
        [np.asarray(res.results[i]["y"]).astype(np.float32) for i in range(NCORES)],
        axis=0,
    )
    return out.reshape(B, C, H, W)


if __name__ == "__main__":
    xs = np.random.randn(B, C, H, W).astype(np.float32)
    got = kernel(xs, 52)
    exp = np.maximum(xs - xs.mean(axis=1, keepdims=True), 0.0)
    err = np.abs(got - exp).max()
    print("abs err vs numpy:", err, " rel:", err / np.abs(exp).max())


# revision 17
# speedup vs baseline: 1.1490x; 1.1490x over previous
"""KWinnersCompetition forward kernel for 8 Trainium2 NeuronCores.

The reference's top-k mask only gates gradients (where(mask, x, stop_grad(x))
has forward value x), so the forward output is exactly:

    out[b, c, h, w] = relu(x[b, c, h, w] - mean_c' x[b, c', h, w])

Sharding: data-parallel over batch. 64 batches / 8 cores = 8 per core,
no communication.

The kernel is purely memory-bound (roofline = HBM traffic / ~430 GB/s
per core), and the tolerance is 2e-2, so the single biggest lever is
moving bf16 instead of f32 across HBM: the host downcasts x to bf16
before upload and upcasts y back to f32 after download, halving the
mandatory traffic (25.7 MB -> 12.85 MB per core). bf16 rounding of x
costs ~2^-9 relative error (~2e-3 of the output max) - an order of
magnitude inside tolerance. It also makes the PE mean input bf16
natively, so no cast op is needed on any engine.

Per-core layout (x shard [8, 512, 784] bf16, C-major so HW is
contiguous). Channels are interleaved onto partitions as c = 4p + j
(partition p, free-dim j in 0..3) so every partition's DMA run is
contiguous DRAM.

DMA plan: ALL transfers (16 half-batch loads, then 16 half-batch
stores) are issued from the Sync engine onto its single HWDGE ring.
Ring FIFO order = issue order, so every load descriptor drains before
any store descriptor: loads get strict priority (every load is on the
critical path of downstream compute; stores only gate the very end).
Earlier two-ring variants let the store ring steal SDMA bandwidth
mid-stream, making the last loads dribble out ~6 us late. Store
dma_starts wait on their relu sems on the otherwise-idle Sync
sequencer, so they can never block compute engines either.

Compute per batch (halves of 392 columns = one PSUM bank):
  - PE:  per half, 4 accumulating bf16 matmuls with a constant 1/512
    weight tile: m = (1/512) * sum_c x[c, :] broadcast to all 128
    partitions (f32 PSUM accumulate). The j0/j1 matmuls only need the
    first half-batch load, so PE starts ~2 us earlier than with
    full-batch loads.
  - ACT: copy the f32 PSUM mean to a bf16 SBUF tile (its only job).
  - DVE: one all-bf16 tensor_sub per half with the mean AP broadcast
    over j (all-bf16 hits DVE's 2x mode, 0.97 us vs 1.78 mixed), then
    relu as all-bf16 tensor_scalar_max (4x mode, ~0.4 us — the ACT
    activation path is 1x and at 1.6 us/half was the pipeline pacer).
"""

import sys

if "/opt/trn_rl_repo" not in sys.path:
    sys.path.insert(0, "/opt/trn_rl_repo")

import numpy as np

B, C, H, W = 64, 512, 28, 28
HW = H * W              # 784
NCORES = 8
BPC = B // NCORES       # 8 batches per core
P = 128                 # partitions
J = C // P              # 4 channels interleaved per partition
HALF = HW // 2          # 392 (matmul free dim <= 512 / one PSUM bank)

_built = None


def _build():
    import concourse.bacc as bacc
    import concourse.bass as bass
    import concourse.tile as tile
    from concourse import mybir

    nc = bacc.Bacc("TRN2", target_bir_lowering=False, debug=False)
    x = nc.dram_tensor("x", [BPC, C, HW], mybir.dt.bfloat16, kind="ExternalInput")
    y = nc.dram_tensor("y", [BPC, C, HW], mybir.dt.bfloat16, kind="ExternalOutput")

    bf16 = mybir.dt.bfloat16

    with tile.TileContext(nc) as tc:
        with (
            tc.tile_pool(name="singles", bufs=1) as singles,
            tc.tile_pool(name="xin", bufs=BPC) as xin,
            tc.tile_pool(name="diffs", bufs=6) as diffs,
            tc.tile_pool(name="outs", bufs=6) as outs,
            tc.tile_pool(name="m16s", bufs=4) as m16s,
            tc.tile_pool(name="means", bufs=4, space="PSUM") as means,
        ):
            wones = singles.tile([P, P], bf16)
            nc.vector.memset(wones, 1.0 / C)

            # 16 half-batch loads (j-pairs, contiguous per partition),
            # all on the Sync ring ahead of every store
            xts = []
            for b in range(BPC):
                xb = x[b].rearrange("(p j) w -> p j w", j=J)
                xt = xin.tile([P, J, HW], bf16)
                nc.sync.dma_start(out=xt[:, 0:2, :], in_=xb[:, 0:2, :])
                nc.sync.dma_start(out=xt[:, 2:4, :], in_=xb[:, 2:4, :])
                xts.append(xt)

            for b in range(BPC):
                yb = y[b].rearrange("(p j) w -> p j w", j=J)
                xt = xts[b]

                dt = diffs.tile([P, J, HW], bf16)
                ot = outs.tile([P, J, HW], bf16)

                for h in range(2):
                    lo = h * HALF
                    hi = lo + HALF
                    m = means.tile([P, HALF], mybir.dt.float32)
                    for j in range(J):
                        nc.tensor.matmul(
                            m,
                            wones,
                            xt[:, j, lo:hi],
                            start=(j == 0),
                            stop=(j == J - 1),
                        )
                    m16 = m16s.tile([P, HALF], bf16)
                    nc.scalar.activation(
                        out=m16, in_=m, func=mybir.ActivationFunctionType.Copy
                    )
                    # mean AP broadcast across the j dim (step 0)
                    map_ = m16[:]
                    m_bcast = bass.AP(
                        tensor=map_.tensor,
                        offset=map_.offset,
                        ap=[map_.ap[0], [0, J], map_.ap[1]],
                    )
                    nc.vector.tensor_sub(dt[:, :, lo:hi], xt[:, :, lo:hi], m_bcast)
                    nc.vector.tensor_scalar_max(ot[:, :, lo:hi], dt[:, :, lo:hi], 0.0)

                # per-batch store (contiguous per partition), issued from
                # Sync: queues on the same ring BEHIND all loads -> loads
                # drain first
                nc.sync.dma_start(out=yb, in_=ot)

    nc.compile()
    return nc


def _get_nc():
    global _built
    if _built is None:
        _built = _build()
    return _built


def _shard(x_full):
    import ml_dtypes

    xf = np.asarray(x_full).reshape(B, C, HW).astype(ml_dtypes.bfloat16)
    return [
        {"x": np.ascontiguousarray(xf[i * BPC : (i + 1) * BPC])}
        for i in range(NCORES)
    ]


def _run(in_maps, **kw):
    from concourse.bass_utils import run_bass_kernel_spmd

    return run_bass_kernel_spmd(_get_nc(), in_maps, list(range(NCORES)), **kw)


def kernel(x, k=None, **_unused):
    res = _run(_shard(np.asarray(x)))
    out = np.concatenate(
        [np.asarray(res.results[i]["y"]).astype(np.float32) for i in range(NCORES)],
        axis=0,
    )
    return out.reshape(B, C, H, W)


if __name__ == "__main__":
    xs = np.random.randn(B, C, H, W).astype(np.float32)
    got = kernel(xs, 52)
    exp = np.maximum(xs - xs.mean(axis=1, keepdims=True), 0.0)
    err = np.abs(got - exp).max()
    print("abs err vs numpy:", err, " rel:", err / np.abs(exp).max())
